# revision 11
# baseline (speedup 1.0000x reference)
"""Fused ACNet-style 5-branch conv block as a single 3x3 conv on Trainium2.

The reference computes
    out = conv3x3(x, w_square) + conv3x1(x, w_ver) + conv1x3(x, w_hor)
        + conv3x3(x, w_diag19 * eye3) + conv3x3(x, w_diag37 * antieye3)
All five branches are linear convs with identical output geometry, so they
fold into ONE effective 3x3 conv whose weight is the sum of the embedded /
masked branch weights.  The conv runs as 9 shifted matmuls (one per tap)
accumulated in PSUM, channels on the 128 SBUF partitions (C_in = C_out = 128):
    out[:, h, w] += W[kh,kw].T @ x_pad[:, h+kh, w+kw]

Input layout: spacer-packed rows — each padded row is 193 elements (192 data
+ 1 shared zero spacer).  The spacer acts as right-pad of row r AND left-pad
of row r+1, so every tap shift is a pure flat offset and each matmul's moving
operand is ONE contiguous 386-element run (2 output rows per PSUM bank).

Operands are bf16 (host-converted): enables fast weight load (FWL) so the
per-matmul 128-col weight load is ~53ns instead of ~107ns fp32, and halves
HBM traffic.  Matmuls are issued tap-major across a set of 4 PSUM banks so
consecutive matmuls share the stationary weights.  PSUM accumulates fp32;
outputs drain to bf16 and are converted back to fp32 on the host.

Sharding: data-parallel over batch — 16 images / 8 cores = 2 images per
core, weights replicated, no collectives.
"""

import sys

for _p in ("/opt/trn_rl_repo",):
    if _p not in sys.path:
        sys.path.insert(0, _p)

import numpy as np

import concourse.mybir as mybir
import concourse.tile as tile
from concourse import bacc
from concourse.bass_utils import run_bass_kernel_spmd
from concourse.tile_rust import add_dep_helper

B, C, H, W = 16, 128, 192, 192
NCORES = 8
IPC = B // NCORES  # images per core
NTAP = 9
SW = W + 1  # spacer-packed row width (193)
XLEN = 1 + (H + 2) * SW + 4  # leading zero + 194 packed rows + tap margin
RB = 32  # output rows per block
GSET = 4  # 2-row groups per PSUM bank set (tap-major inner tile)
MM_DT = mybir.dt.bfloat16

_bf16 = None


def _np_bf16():
    global _bf16
    if _bf16 is None:
        _bf16 = mybir.dt.np(mybir.dt.bfloat16)
    return _bf16


def _mm_noldw(nc, out, lhsT, rhs, start, stop):
    """InstMatmult with ldweights=False: uses the weights already loaded
    into the PE array by a preceding explicit nc.tensor.ldweights."""
    te = nc.tensor
    ifmap_ap = te.lower_ap(rhs.opt({0}), opt=False)
    weights_ap = te.lower_ap(lhsT.opt({0}), opt=False, for_matmul_weights=True)
    out_ap = te.lower_ap(out)
    return te.add_instruction(
        mybir.InstMatmult(
            name=te.bass.get_next_instruction_name(),
            replication_resolution=0,
            replication_shift_amnt=0,
            replication_num_rows=0,
            start_tensor_calc=start,
            stop_tensor_calc=stop,
            ins=[ifmap_ap, weights_ap],
            outs=[out_ap],
            perf_mode=None,
            is_transpose=None,
            ifmap_quant_offset=None,
            weights_quant_offset=None,
            bass_skip_group_check=False,
            tile_position=(0, 0),
            tile_size=(128, 128),
            ldweights=False,
        )
    )


def _dedupe_ldweights(nc):
    """Drop InstLdweights that reload the weights already resident in the
    PE array (identical AP to the previous load, no different load in
    between).  The compile pipeline emits one LDWEIGHTS per matmul even for
    ldweights=False matmuls; within a tap-major run they are redundant.
    Waits/updates on a dropped LDW migrate to the next kept PE instruction.
    """
    for bb in nc.m.functions[0].blocks:
        il = bb.instructions
        keep = []
        last_key = None
        for inst in il:
            if isinstance(inst, mybir.InstLdweights):
                key = str(inst.ins[0])
                si = inst.sync_info
                has_sync = si is not None and (
                    len(si.on_wait) > 0 or len(si.on_update) > 0
                )
                if key == last_key and not has_sync:
                    continue  # redundant reload with no sync riding: drop
                last_key = key
            keep.append(inst)
        il[:] = keep


def _build(ipc, rb, mm_dt, repeat=1, xbufs=3, obufs=2, ahead=1, gset=GSET):
    """Emit the per-core Bass program.

    The x-DMA for block k+ahead is issued before block k's compute/out-DMA
    in program order, so input prefetch never queues behind output drains.
    repeat>1 wraps the body in a For_i loop (timing harness only; the body
    is idempotent so outputs are unchanged).
    """
    nc = bacc.Bacc("TRN2", target_bir_lowering=False, debug=False)
    x_in = nc.dram_tensor(
        "x", [ipc, C, XLEN], mm_dt, kind="ExternalInput"
    ).ap()
    w_in = nc.dram_tensor(
        "w", [C, NTAP * C], mm_dt, kind="ExternalInput"
    ).ap()
    y_out = nc.dram_tensor(
        "y", [ipc, C, H, W], mm_dt, kind="ExternalOutput"
    ).ap()

    xtl = (rb + 2) * SW + 4  # x tile flat length per partition
    blocks = [(img, r0) for img in range(ipc) for r0 in range(0, H, rb)]

    with tile.TileContext(nc) as tc:
        with (
            tc.tile_pool(name="wp", bufs=1) as wpool,
            tc.tile_pool(name="xp", bufs=xbufs) as xpool,
            tc.tile_pool(name="op", bufs=obufs) as opool,
            tc.tile_pool(name="ps", bufs=8, space="PSUM") as pspool,
        ):
            wt = wpool.tile([C, NTAP * C], mm_dt)
            nc.sync.dma_start(wt[:], w_in[:])

            def load(img, r0):
                xt = xpool.tile([C, xtl], mm_dt, tag="xt", name=f"xt{img}_{r0}")
                base = r0 * SW
                nc.sync.dma_start(xt[:], x_in[img, :, base : base + xtl])
                return xt

            prev_pe = [None]  # last PE instruction, for the order chain

            def chain(bi):
                # Serialize the PE stream in program order: the scheduler
                # can't see the PE weight-state hazard between a standalone
                # ldweights and the non-self-loading matmuls that use it.
                if prev_pe[0] is not None:
                    add_dep_helper(bi.ins, prev_pe[0].ins, sync=False,
                                   reason="pe-weight-state order")
                prev_pe[0] = bi

            def body():
                xts = [load(*blocks[k]) for k in range(min(ahead, len(blocks)))]
                for k, (img, r0) in enumerate(blocks):
                    if k + ahead < len(blocks):
                        xts.append(load(*blocks[k + ahead]))
                    xt = xts.pop(0)
                    ot = opool.tile([C, rb, W], mm_dt, tag="ot",
                                    name=f"ot{img}_{r0}")
                    ngroups = rb // 2
                    for s in range(0, ngroups, gset):
                        nset = min(gset, ngroups - s)
                        pss = [
                            pspool.tile([C, 2, SW], mybir.dt.float32,
                                        tag="ps", name=f"ps{s + i}")
                            for i in range(nset)
                        ]
                        # tap-major with one explicit weight load per tap,
                        # amortized over the whole bank set.
                        for t in range(NTAP):
                            kh, kw = divmod(t, 3)
                            chain(nc.tensor.ldweights(
                                wt[:, t * C : (t + 1) * C]))
                            for i in range(nset):
                                p = s + i
                                off = (2 * p + kh) * SW + kw
                                chain(_mm_noldw(
                                    nc,
                                    pss[i][:],
                                    wt[:, t * C : (t + 1) * C],
                                    xt[:, off : off + 2 * SW],
                                    start=(t == 0),
                                    stop=(t == NTAP - 1),
                                ))
                        # strip the spacer columns while draining PSUM
                        # (one strided 2-row op per bank)
                        for i in range(nset):
                            p = s + i
                            eng = nc.scalar.copy if i % 2 == 0 else (
                                nc.vector.tensor_copy
                            )
                            eng(ot[:, 2 * p : 2 * p + 2, :],
                                pss[i][:, :, 0:W])
                    nc.sync.dma_start(y_out[img, :, r0 : r0 + rb, :], ot[:])

            if repeat == 1:
                body()
            else:
                with tc.For_i(0, repeat, 1):
                    body()
    nc.compile()
    _dedupe_ldweights(nc)
    return nc


def _fold_weights(w_square, w_ver, w_hor, w_diag19, w_diag37):
    """Fold the 5 branches into one 3x3 weight, laid out [C_in, tap*C_out]."""
    eye = np.eye(3, dtype=np.float32)
    anti = eye[::-1, :]
    w_eff = (
        np.asarray(w_square, np.float32)
        + np.asarray(w_diag19, np.float32) * eye
        + np.asarray(w_diag37, np.float32) * anti
    )
    w_eff[:, :, :, 1] += np.asarray(w_ver, np.float32)[:, :, :, 0]
    w_eff[:, :, 1, :] += np.asarray(w_hor, np.float32)[:, :, 0, :]
    # [O, I, KH, KW] -> [I, KH, KW, O] -> [I, (KH*KW)*O]  (lhsT per tap)
    w = np.ascontiguousarray(w_eff.transpose(1, 2, 3, 0).reshape(C, NTAP * C))
    return w.astype(_np_bf16())


def _pack_x(x):
    """[B,C,H,W] -> spacer-packed flat bf16 [B,C,XLEN]."""
    xs = np.zeros((B, C, XLEN), _np_bf16())
    rows = xs[:, :, 1 : 1 + (H + 2) * SW].reshape(B, C, H + 2, SW)
    rows[:, :, 1 : H + 1, 0:W] = x.astype(_np_bf16())
    return xs


_nc_cache = {}


def kernel(x, w_square, w_ver, w_hor, w_diag19, w_diag37):
    x = np.asarray(x, np.float32)
    w_host = _fold_weights(w_square, w_ver, w_hor, w_diag19, w_diag37)
    xs = _pack_x(x)

    if "nc" not in _nc_cache:
        _nc_cache["nc"] = _build(IPC, RB, MM_DT)
    nc = _nc_cache["nc"]

    in_maps = [
        {"x": np.ascontiguousarray(xs[c * IPC : (c + 1) * IPC]), "w": w_host}
        for c in range(NCORES)
    ]
    res = run_bass_kernel_spmd(nc, in_maps, list(range(NCORES)))
    out = np.concatenate([res.results[c]["y"] for c in range(NCORES)], axis=0)
    return out.astype(np.float32)


# revision 21
# speedup vs baseline: 1.0112x; 1.0112x over previous
"""Fused ACNet-style 5-branch conv block as a single 3x3 conv on Trainium2.

The reference computes
    out = conv3x3(x, w_square) + conv3x1(x, w_ver) + conv1x3(x, w_hor)
        + conv3x3(x, w_diag19 * eye3) + conv3x3(x, w_diag37 * antieye3)
All five branches are linear convs with identical output geometry, so they
fold into ONE effective 3x3 conv whose weight is the sum of the embedded /
masked branch weights.  The conv runs as 9 shifted matmuls (one per tap)
accumulated in PSUM, channels on the 128 SBUF partitions (C_in = C_out = 128):
    out[:, h, w] += W[kh,kw].T @ x_pad[:, h+kh, w+kw]

Input layout: spacer-packed rows — each padded row is 193 elements (192 data
+ 1 shared zero spacer).  The spacer acts as right-pad of row r AND left-pad
of row r+1, so every tap shift is a pure flat offset and each matmul's moving
operand is ONE contiguous 386-element run (2 output rows per PSUM bank).

Operands are bf16 (host-converted): enables fast weight load (FWL) so the
per-matmul 128-col weight load is ~53ns instead of ~107ns fp32, and halves
HBM traffic.  Matmuls are issued tap-major across a set of 4 PSUM banks so
consecutive matmuls share the stationary weights.  PSUM accumulates fp32;
outputs drain to bf16 and are converted back to fp32 on the host.

Sharding: data-parallel over batch — 16 images / 8 cores = 2 images per
core, weights replicated, no collectives.
"""

import sys

for _p in ("/opt/trn_rl_repo",):
    if _p not in sys.path:
        sys.path.insert(0, _p)

import numpy as np

import concourse.mybir as mybir
import concourse.tile as tile
from concourse import bacc
from concourse.bass_utils import run_bass_kernel_spmd
from concourse.tile_rust import add_dep_helper

B, C, H, W = 16, 128, 192, 192
NCORES = 8
IPC = B // NCORES  # images per core
NTAP = 9
SW = W + 1  # spacer-packed row width (193)
XLEN = 1 + (H + 2) * SW + 4  # leading zero + 194 packed rows + tap margin
RB = 32  # output rows per block
GSET = 4  # 2-row groups per PSUM bank set (tap-major inner tile)
MM_DT = mybir.dt.bfloat16

_bf16 = None


def _np_bf16():
    global _bf16
    if _bf16 is None:
        _bf16 = mybir.dt.np(mybir.dt.bfloat16)
    return _bf16


def _mm_noldw(nc, out, lhsT, rhs, start, stop):
    """InstMatmult with ldweights=False: uses the weights already loaded
    into the PE array by a preceding explicit nc.tensor.ldweights."""
    te = nc.tensor
    ifmap_ap = te.lower_ap(rhs.opt({0}), opt=False)
    weights_ap = te.lower_ap(lhsT.opt({0}), opt=False, for_matmul_weights=True)
    out_ap = te.lower_ap(out)
    return te.add_instruction(
        mybir.InstMatmult(
            name=te.bass.get_next_instruction_name(),
            replication_resolution=0,
            replication_shift_amnt=0,
            replication_num_rows=0,
            start_tensor_calc=start,
            stop_tensor_calc=stop,
            ins=[ifmap_ap, weights_ap],
            outs=[out_ap],
            perf_mode=None,
            is_transpose=None,
            ifmap_quant_offset=None,
            weights_quant_offset=None,
            bass_skip_group_check=False,
            tile_position=(0, 0),
            tile_size=(128, 128),
            ldweights=False,
        )
    )


def _dedupe_ldweights(nc):
    """Drop InstLdweights that reload the weights already resident in the
    PE array (identical AP to the previous load, no different load in
    between).  The compile pipeline emits one LDWEIGHTS per matmul even for
    ldweights=False matmuls; within a tap-major run they are redundant.
    Waits/updates on a dropped LDW migrate to the next kept PE instruction.
    """
    for bb in nc.m.functions[0].blocks:
        il = bb.instructions
        keep = []
        last_key = None
        for inst in il:
            if isinstance(inst, mybir.InstLdweights):
                key = str(inst.ins[0])
                si = inst.sync_info
                has_sync = si is not None and (
                    len(si.on_wait) > 0 or len(si.on_update) > 0
                )
                if key == last_key and not has_sync:
                    continue  # redundant reload with no sync riding: drop
                last_key = key
            keep.append(inst)
        il[:] = keep


def _build(ipc, rb, mm_dt, repeat=1, xbufs=3, obufs=2, ahead=1, gset=GSET,
           do_mm=True, do_in=True, do_out=True, do_drain=True, psum2d=True):
    """Emit the per-core Bass program.

    The x-DMA for block k+ahead is issued before block k's compute/out-DMA
    in program order, so input prefetch never queues behind output drains.
    repeat>1 wraps the body in a For_i loop (timing harness only; the body
    is idempotent so outputs are unchanged).
    """
    nc = bacc.Bacc("TRN2", target_bir_lowering=False, debug=False)
    x_in = nc.dram_tensor(
        "x", [ipc, C, XLEN], mm_dt, kind="ExternalInput"
    ).ap()
    w_in = nc.dram_tensor(
        "w", [C, NTAP * C], mm_dt, kind="ExternalInput"
    ).ap()
    y_out = nc.dram_tensor(
        "y", [ipc, C, H, W], mm_dt, kind="ExternalOutput"
    ).ap()

    xtl = (rb + 2) * SW + 4  # x tile flat length per partition
    blocks = [(img, r0) for img in range(ipc) for r0 in range(0, H, rb)]

    with tile.TileContext(nc) as tc:
        with (
            tc.tile_pool(name="wp", bufs=1) as wpool,
            tc.tile_pool(name="xp", bufs=xbufs) as xpool,
            tc.tile_pool(name="op", bufs=obufs) as opool,
            tc.tile_pool(name="ps", bufs=8, space="PSUM") as pspool,
        ):
            wt = wpool.tile([C, NTAP * C], mm_dt)
            nc.sync.dma_start(wt[:], w_in[:])

            def load(img, r0):
                xt = xpool.tile([C, xtl], mm_dt, tag="xt", name=f"xt{img}_{r0}")
                base = r0 * SW
                if do_in:
                    nc.sync.dma_start(xt[:], x_in[img, :, base : base + xtl])
                return xt

            prev_pe = [None]  # last PE instruction, for the order chain

            def chain(bi):
                # Serialize the PE stream in program order: the scheduler
                # can't see the PE weight-state hazard between a standalone
                # ldweights and the non-self-loading matmuls that use it.
                if prev_pe[0] is not None:
                    add_dep_helper(bi.ins, prev_pe[0].ins, sync=False,
                                   reason="pe-weight-state order")
                prev_pe[0] = bi

            def body():
                xts = [load(*blocks[k]) for k in range(min(ahead, len(blocks)))]
                for k, (img, r0) in enumerate(blocks):
                    if k + ahead < len(blocks):
                        xts.append(load(*blocks[k + ahead]))
                    xt = xts.pop(0)
                    ot = opool.tile([C, rb, W], mm_dt, tag="ot",
                                    name=f"ot{img}_{r0}")
                    ngroups = rb // 2
                    for s in range(0, ngroups, gset):
                        nset = min(gset, ngroups - s)
                        shape = [C, 2, SW] if psum2d else [C, 2 * SW]
                        pss = [
                            pspool.tile(shape, mybir.dt.float32,
                                        tag="ps", name=f"ps{s + i}")
                            for i in range(nset)
                        ]
                        # tap-major with one explicit weight load per tap,
                        # amortized over the whole bank set.
                        if do_mm:
                            for t in range(NTAP):
                                kh, kw = divmod(t, 3)
                                chain(nc.tensor.ldweights(
                                    wt[:, t * C : (t + 1) * C]))
                                for i in range(nset):
                                    p = s + i
                                    off = (2 * p + kh) * SW + kw
                                    chain(_mm_noldw(
                                        nc,
                                        pss[i][:],
                                        wt[:, t * C : (t + 1) * C],
                                        xt[:, off : off + 2 * SW],
                                        start=(t == 0),
                                        stop=(t == NTAP - 1),
                                    ))
                        # strip the spacer columns while draining PSUM
                        # (one strided 2-row op per bank)
                        if do_drain and do_mm:
                            for i in range(nset):
                                p = s + i
                                eng = nc.scalar.copy if i % 2 == 0 else (
                                    nc.vector.tensor_copy
                                )
                                if psum2d:
                                    eng(ot[:, 2 * p : 2 * p + 2, :],
                                        pss[i][:, :, 0:W])
                                else:
                                    eng(ot[:, 2 * p, :], pss[i][:, 0:W])
                                    eng(ot[:, 2 * p + 1, :],
                                        pss[i][:, SW : SW + W])
                    if do_out:
                        nc.sync.dma_start(
                            y_out[img, :, r0 : r0 + rb, :], ot[:])

            if repeat == 1:
                body()
            else:
                with tc.For_i(0, repeat, 1):
                    body()
    nc.compile()
    _dedupe_ldweights(nc)
    return nc


FLAT = H * SW  # packed output length per image (192 rows x 193)
CHUNK = 512  # matmul free dim / PSUM bank
XGRP = 16  # chunks per x-load group
PSET = 4  # chunks per PSUM bank set (tap-major inner tile)


def _chunks_of(img_len):
    """Chunk starts+lengths covering [0, img_len) in 512s + remainder."""
    out = []
    c0 = 0
    while c0 < img_len:
        out.append((c0, min(CHUNK, img_len - c0)))
        c0 += CHUNK
    return out


def _build_flat(ipc, repeat=1, xbufs=3, obufs=2, ahead=1, pset=PSET,
                mm_dt=MM_DT):
    """Flat-chunk variant: the packed per-image output space [0, H*SW) is
    tiled into 512-wide matmul chunks (full PSUM banks), ignoring row
    boundaries — a tap shift is a pure flat offset, so any flat window is a
    valid moving operand.  Outputs stay spacer-packed ([C, H*SW] bf16 in
    DRAM); the host strips the 193rd column of each row.

    x is streamed in groups of 16 chunks (+390-element tap halo), output
    drains into 2-superset (1 MiB) tiles.
    """
    nc = bacc.Bacc("TRN2", target_bir_lowering=False, debug=False)
    x_in = nc.dram_tensor(
        "x", [ipc, C, XLEN], mm_dt, kind="ExternalInput"
    ).ap()
    w_in = nc.dram_tensor(
        "w", [C, NTAP * C], mm_dt, kind="ExternalInput"
    ).ap()
    y_out = nc.dram_tensor(
        "y", [ipc, C, FLAT], mm_dt, kind="ExternalOutput"
    ).ap()

    halo = 2 * SW + 2  # max tap offset reach
    chunks = _chunks_of(FLAT)
    groups = [chunks[g : g + XGRP] for g in range(0, len(chunks), XGRP)]
    loads = []  # (img, g0, glen_x)
    for img in range(ipc):
        for grp in groups:
            g0 = grp[0][0]
            gend = grp[-1][0] + grp[-1][1]
            loads.append((img, g0, min(gend + halo, XLEN) - g0, grp))

    with tile.TileContext(nc) as tc:
        with (
            tc.tile_pool(name="wp", bufs=1) as wpool,
            tc.tile_pool(name="xp", bufs=xbufs) as xpool,
            tc.tile_pool(name="op", bufs=obufs) as opool,
            tc.tile_pool(name="ps", bufs=8, space="PSUM") as pspool,
        ):
            wt = wpool.tile([C, NTAP * C], mm_dt)
            nc.sync.dma_start(wt[:], w_in[:])

            xtl = XGRP * CHUNK + halo
            prev_pe = [None]

            def chain(bi):
                if prev_pe[0] is not None:
                    add_dep_helper(bi.ins, prev_pe[0].ins, sync=False,
                                   reason="pe-weight-state order")
                prev_pe[0] = bi

            def load(k):
                img, g0, xl, grp = loads[k]
                xt = xpool.tile([C, xtl], mm_dt, tag="xt", name=f"xt{k}")
                nc.sync.dma_start(xt[:, 0:xl], x_in[img, :, g0 : g0 + xl])
                return xt

            def body():
                xts = [load(k) for k in range(min(ahead, len(loads)))]
                for k, (img, g0, xl, grp) in enumerate(loads):
                    if k + ahead < len(loads):
                        xts.append(load(k + ahead))
                    xt = xts.pop(0)
                    glen = grp[-1][0] + grp[-1][1] - g0
                    ot = opool.tile([C, XGRP * CHUNK], mm_dt, tag="ot",
                                    name=f"ot{k}")
                    for s in range(0, len(grp), pset):
                        sset = grp[s : s + pset]
                        pss = [
                            pspool.tile([C, CHUNK], mybir.dt.float32,
                                        tag="ps", name=f"ps{k}_{s + i}")
                            for i in range(len(sset))
                        ]
                        for t in range(NTAP):
                            kh, kw = divmod(t, 3)
                            toff = kh * SW + kw
                            chain(nc.tensor.ldweights(
                                wt[:, t * C : (t + 1) * C]))
                            for i, (c0, cl) in enumerate(sset):
                                loc = c0 - g0 + toff
                                chain(_mm_noldw(
                                    nc,
                                    pss[i][:, 0:cl],
                                    wt[:, t * C : (t + 1) * C],
                                    xt[:, loc : loc + cl],
                                    start=(t == 0),
                                    stop=(t == NTAP - 1),
                                ))
                        for i, (c0, cl) in enumerate(sset):
                            eng = nc.scalar.copy if i % 2 == 0 else (
                                nc.vector.tensor_copy
                            )
                            eng(ot[:, c0 - g0 : c0 - g0 + cl],
                                pss[i][:, 0:cl])
                    nc.sync.dma_start(
                        y_out[img, :, g0 : g0 + glen], ot[:, 0:glen]
                    )

            if repeat == 1:
                body()
            else:
                with tc.For_i(0, repeat, 1):
                    body()
    nc.compile()
    _dedupe_ldweights(nc)
    return nc


def _fold_weights(w_square, w_ver, w_hor, w_diag19, w_diag37):
    """Fold the 5 branches into one 3x3 weight, laid out [C_in, tap*C_out]."""
    eye = np.eye(3, dtype=np.float32)
    anti = eye[::-1, :]
    w_eff = (
        np.asarray(w_square, np.float32)
        + np.asarray(w_diag19, np.float32) * eye
        + np.asarray(w_diag37, np.float32) * anti
    )
    w_eff[:, :, :, 1] += np.asarray(w_ver, np.float32)[:, :, :, 0]
    w_eff[:, :, 1, :] += np.asarray(w_hor, np.float32)[:, :, 0, :]
    # [O, I, KH, KW] -> [I, KH, KW, O] -> [I, (KH*KW)*O]  (lhsT per tap)
    w = np.ascontiguousarray(w_eff.transpose(1, 2, 3, 0).reshape(C, NTAP * C))
    return w.astype(_np_bf16())


def _pack_x(x):
    """[B,C,H,W] -> spacer-packed flat bf16 [B,C,XLEN]."""
    xs = np.zeros((B, C, XLEN), _np_bf16())
    rows = xs[:, :, 1 : 1 + (H + 2) * SW].reshape(B, C, H + 2, SW)
    rows[:, :, 1 : H + 1, 0:W] = x.astype(_np_bf16())
    return xs


_nc_cache = {}


def build_program(repeat=1, **flags):
    """Canonical program used by kernel() and test.py."""
    return _build_flat(IPC, repeat=repeat, **flags)


def _unpack_y(y):
    """[B, C, ...] device output -> [B, C, H, W] fp32."""
    y = np.asarray(y)
    if y.shape[-1] == FLAT:  # spacer-packed flat layout
        y = y.reshape(B, C, H, SW)[:, :, :, 0:W]
    return np.ascontiguousarray(y.astype(np.float32))


def kernel(x, w_square, w_ver, w_hor, w_diag19, w_diag37):
    x = np.asarray(x, np.float32)
    w_host = _fold_weights(w_square, w_ver, w_hor, w_diag19, w_diag37)
    xs = _pack_x(x)

    if "nc" not in _nc_cache:
        _nc_cache["nc"] = build_program()
    nc = _nc_cache["nc"]

    in_maps = [
        {"x": np.ascontiguousarray(xs[c * IPC : (c + 1) * IPC]), "w": w_host}
        for c in range(NCORES)
    ]
    res = run_bass_kernel_spmd(nc, in_maps, list(range(NCORES)))
    out = np.concatenate([res.results[c]["y"] for c in range(NCORES)], axis=0)
    return _unpack_y(out)


# revision 26
# speedup vs baseline: 1.0697x; 1.0578x over previous
"""Fused ACNet-style 5-branch conv block as a single 3x3 conv on Trainium2.

The reference computes
    out = conv3x3(x, w_square) + conv3x1(x, w_ver) + conv1x3(x, w_hor)
        + conv3x3(x, w_diag19 * eye3) + conv3x3(x, w_diag37 * antieye3)
All five branches are linear convs with identical output geometry, so they
fold into ONE effective 3x3 conv whose weight is the sum of the embedded /
masked branch weights.  The conv runs as 9 shifted matmuls (one per tap)
accumulated in PSUM, channels on the 128 SBUF partitions (C_in = C_out = 128):
    out[:, h, w] += W[kh,kw].T @ x_pad[:, h+kh, w+kw]

Input layout: spacer-packed rows — each padded row is 193 elements (192 data
+ 1 shared zero spacer).  The spacer acts as right-pad of row r AND left-pad
of row r+1, so every tap shift is a pure flat offset and ANY flat window of
the packed image is a valid moving operand.  The packed output space
(H*193 per image) is tiled into 512-wide chunks = full PSUM banks
(_build_flat), minimizing matmul instruction count (73/image vs 96 for
row-pair tiling).

Operands are bf16 (host-converted): enables fast weight load and halves HBM
traffic; PSUM accumulates fp32, outputs drain to bf16 and the host converts
back to fp32 (rel err ~3.4e-3, tolerance 2e-2).  Matmuls are issued
tap-major across sets of 4 PSUM banks with ONE explicit LDWEIGHTS per tap
(matmuls carry ldweights=False; a post-compile pass drops the redundant
auto-inserted weight reloads — ~70us win at fp32->bf16+dedupe combined).
Tap order zigzags between sets so boundary taps share weight loads.
Outputs stay spacer-packed in DRAM; the host strips the 193rd columns.

Sharding: data-parallel over batch — 16 images / 8 cores = 2 images per
core, weights replicated, no collectives.
"""

import sys

for _p in ("/opt/trn_rl_repo",):
    if _p not in sys.path:
        sys.path.insert(0, _p)

import numpy as np

import concourse.mybir as mybir
import concourse.tile as tile
from concourse import bacc
from concourse.bass_utils import run_bass_kernel_spmd
from concourse.tile_rust import add_dep_helper

B, C, H, W = 16, 128, 192, 192
NCORES = 8
IPC = B // NCORES  # images per core
NTAP = 9
SW = W + 1  # spacer-packed row width (193)
XLEN = 1 + (H + 2) * SW + 4  # leading zero + 194 packed rows + tap margin
RB = 32  # output rows per block
GSET = 4  # 2-row groups per PSUM bank set (tap-major inner tile)
MM_DT = mybir.dt.bfloat16

_bf16 = None


def _np_bf16():
    global _bf16
    if _bf16 is None:
        _bf16 = mybir.dt.np(mybir.dt.bfloat16)
    return _bf16


def _mm_noldw(nc, out, lhsT, rhs, start, stop):
    """InstMatmult with ldweights=False: uses the weights already loaded
    into the PE array by a preceding explicit nc.tensor.ldweights."""
    te = nc.tensor
    ifmap_ap = te.lower_ap(rhs.opt({0}), opt=False)
    weights_ap = te.lower_ap(lhsT.opt({0}), opt=False, for_matmul_weights=True)
    out_ap = te.lower_ap(out)
    return te.add_instruction(
        mybir.InstMatmult(
            name=te.bass.get_next_instruction_name(),
            replication_resolution=0,
            replication_shift_amnt=0,
            replication_num_rows=0,
            start_tensor_calc=start,
            stop_tensor_calc=stop,
            ins=[ifmap_ap, weights_ap],
            outs=[out_ap],
            perf_mode=None,
            is_transpose=None,
            ifmap_quant_offset=None,
            weights_quant_offset=None,
            bass_skip_group_check=False,
            tile_position=(0, 0),
            tile_size=(128, 128),
            ldweights=False,
        )
    )


def _dedupe_ldweights(nc):
    """Drop InstLdweights that reload the weights already resident in the
    PE array (identical AP to the previous load, no different load in
    between).  The compile pipeline emits one LDWEIGHTS per matmul even for
    ldweights=False matmuls; within a tap-major run they are redundant.
    Waits/updates on a dropped LDW migrate to the next kept PE instruction.
    """
    for bb in nc.m.functions[0].blocks:
        il = bb.instructions
        keep = []
        last_key = None
        for inst in il:
            if isinstance(inst, mybir.InstLdweights):
                key = str(inst.ins[0])
                si = inst.sync_info
                has_sync = si is not None and (
                    len(si.on_wait) > 0 or len(si.on_update) > 0
                )
                if key == last_key and not has_sync:
                    continue  # redundant reload with no sync riding: drop
                last_key = key
            keep.append(inst)
        il[:] = keep


def _build(ipc, rb, mm_dt, repeat=1, xbufs=3, obufs=2, ahead=1, gset=GSET,
           do_mm=True, do_in=True, do_out=True, do_drain=True, psum2d=True):
    """Emit the per-core Bass program.

    The x-DMA for block k+ahead is issued before block k's compute/out-DMA
    in program order, so input prefetch never queues behind output drains.
    repeat>1 wraps the body in a For_i loop (timing harness only; the body
    is idempotent so outputs are unchanged).
    """
    nc = bacc.Bacc("TRN2", target_bir_lowering=False, debug=False)
    x_in = nc.dram_tensor(
        "x", [ipc, C, XLEN], mm_dt, kind="ExternalInput"
    ).ap()
    w_in = nc.dram_tensor(
        "w", [C, NTAP * C], mm_dt, kind="ExternalInput"
    ).ap()
    y_out = nc.dram_tensor(
        "y", [ipc, C, H, W], mm_dt, kind="ExternalOutput"
    ).ap()

    xtl = (rb + 2) * SW + 4  # x tile flat length per partition
    blocks = [(img, r0) for img in range(ipc) for r0 in range(0, H, rb)]

    with tile.TileContext(nc) as tc:
        with (
            tc.tile_pool(name="wp", bufs=1) as wpool,
            tc.tile_pool(name="xp", bufs=xbufs) as xpool,
            tc.tile_pool(name="op", bufs=obufs) as opool,
            tc.tile_pool(name="ps", bufs=8, space="PSUM") as pspool,
        ):
            wt = wpool.tile([C, NTAP * C], mm_dt)
            nc.sync.dma_start(wt[:], w_in[:])

            def load(img, r0):
                xt = xpool.tile([C, xtl], mm_dt, tag="xt", name=f"xt{img}_{r0}")
                base = r0 * SW
                if do_in:
                    nc.sync.dma_start(xt[:], x_in[img, :, base : base + xtl])
                return xt

            prev_pe = [None]  # last PE instruction, for the order chain

            def chain(bi):
                # Serialize the PE stream in program order: the scheduler
                # can't see the PE weight-state hazard between a standalone
                # ldweights and the non-self-loading matmuls that use it.
                if prev_pe[0] is not None:
                    add_dep_helper(bi.ins, prev_pe[0].ins, sync=False,
                                   reason="pe-weight-state order")
                prev_pe[0] = bi

            def body():
                xts = [load(*blocks[k]) for k in range(min(ahead, len(blocks)))]
                for k, (img, r0) in enumerate(blocks):
                    if k + ahead < len(blocks):
                        xts.append(load(*blocks[k + ahead]))
                    xt = xts.pop(0)
                    ot = opool.tile([C, rb, W], mm_dt, tag="ot",
                                    name=f"ot{img}_{r0}")
                    ngroups = rb // 2
                    for s in range(0, ngroups, gset):
                        nset = min(gset, ngroups - s)
                        shape = [C, 2, SW] if psum2d else [C, 2 * SW]
                        pss = [
                            pspool.tile(shape, mybir.dt.float32,
                                        tag="ps", name=f"ps{s + i}")
                            for i in range(nset)
                        ]
                        # tap-major with one explicit weight load per tap,
                        # amortized over the whole bank set.
                        if do_mm:
                            for t in range(NTAP):
                                kh, kw = divmod(t, 3)
                                chain(nc.tensor.ldweights(
                                    wt[:, t * C : (t + 1) * C]))
                                for i in range(nset):
                                    p = s + i
                                    off = (2 * p + kh) * SW + kw
                                    chain(_mm_noldw(
                                        nc,
                                        pss[i][:],
                                        wt[:, t * C : (t + 1) * C],
                                        xt[:, off : off + 2 * SW],
                                        start=(t == 0),
                                        stop=(t == NTAP - 1),
                                    ))
                        # strip the spacer columns while draining PSUM
                        # (one strided 2-row op per bank)
                        if do_drain and do_mm:
                            for i in range(nset):
                                p = s + i
                                eng = nc.scalar.copy if i % 2 == 0 else (
                                    nc.vector.tensor_copy
                                )
                                if psum2d:
                                    eng(ot[:, 2 * p : 2 * p + 2, :],
                                        pss[i][:, :, 0:W])
                                else:
                                    eng(ot[:, 2 * p, :], pss[i][:, 0:W])
                                    eng(ot[:, 2 * p + 1, :],
                                        pss[i][:, SW : SW + W])
                    if do_out:
                        nc.sync.dma_start(
                            y_out[img, :, r0 : r0 + rb, :], ot[:])

            if repeat == 1:
                body()
            else:
                with tc.For_i(0, repeat, 1):
                    body()
    nc.compile()
    _dedupe_ldweights(nc)
    return nc


FLAT = H * SW  # packed output length per image (192 rows x 193)
CHUNK = 512  # matmul free dim / PSUM bank
XGRP = 16  # chunks per x-load group
PSET = 4  # chunks per PSUM bank set (tap-major inner tile)


def _chunks_of(img_len):
    """Chunk starts+lengths covering [0, img_len) in 512s + remainder."""
    out = []
    c0 = 0
    while c0 < img_len:
        out.append((c0, min(CHUNK, img_len - c0)))
        c0 += CHUNK
    return out


def _build_flat(ipc, repeat=1, xbufs=3, obufs=2, ahead=1, pset=PSET,
                mm_dt=MM_DT):
    """Flat-chunk variant: the packed per-image output space [0, H*SW) is
    tiled into 512-wide matmul chunks (full PSUM banks), ignoring row
    boundaries — a tap shift is a pure flat offset, so any flat window is a
    valid moving operand.  Outputs stay spacer-packed ([C, H*SW] bf16 in
    DRAM); the host strips the 193rd column of each row.

    x is streamed in groups of 16 chunks (+390-element tap halo), output
    drains into 2-superset (1 MiB) tiles.
    """
    nc = bacc.Bacc("TRN2", target_bir_lowering=False, debug=False)
    x_in = nc.dram_tensor(
        "x", [ipc, C, XLEN], mm_dt, kind="ExternalInput"
    ).ap()
    w_in = nc.dram_tensor(
        "w", [C, NTAP * C], mm_dt, kind="ExternalInput"
    ).ap()
    y_out = nc.dram_tensor(
        "y", [ipc, C, FLAT], mm_dt, kind="ExternalOutput"
    ).ap()

    halo = 2 * SW + 2  # max tap offset reach
    chunks = _chunks_of(FLAT)
    groups = [chunks[g : g + XGRP] for g in range(0, len(chunks), XGRP)]
    loads = []  # (img, g0, glen_x)
    for img in range(ipc):
        for grp in groups:
            g0 = grp[0][0]
            gend = grp[-1][0] + grp[-1][1]
            loads.append((img, g0, min(gend + halo, XLEN) - g0, grp))

    with tile.TileContext(nc) as tc:
        with (
            tc.tile_pool(name="wp", bufs=1) as wpool,
            tc.tile_pool(name="xp", bufs=xbufs) as xpool,
            tc.tile_pool(name="op", bufs=obufs) as opool,
            tc.tile_pool(name="ps", bufs=8, space="PSUM") as pspool,
        ):
            wt = wpool.tile([C, NTAP * C], mm_dt)
            nc.sync.dma_start(wt[:], w_in[:])

            xtl = XGRP * CHUNK + halo
            prev_pe = [None]

            def chain(bi):
                if prev_pe[0] is not None:
                    add_dep_helper(bi.ins, prev_pe[0].ins, sync=False,
                                   reason="pe-weight-state order")
                prev_pe[0] = bi

            def load(k):
                img, g0, xl, grp = loads[k]
                xt = xpool.tile([C, xtl], mm_dt, tag="xt", name=f"xt{k}")
                nc.sync.dma_start(xt[:, 0:xl], x_in[img, :, g0 : g0 + xl])
                return xt

            nset_state = [0]

            def body():
                xts = [load(k) for k in range(min(ahead, len(loads)))]
                for k, (img, g0, xl, grp) in enumerate(loads):
                    if k + ahead < len(loads):
                        xts.append(load(k + ahead))
                    xt = xts.pop(0)
                    glen = grp[-1][0] + grp[-1][1] - g0
                    ot = opool.tile([C, XGRP * CHUNK], mm_dt, tag="ot",
                                    name=f"ot{k}")
                    for s in range(0, len(grp), pset):
                        sset = grp[s : s + pset]
                        pss = [
                            pspool.tile([C, CHUNK], mybir.dt.float32,
                                        tag="ps", name=f"ps{k}_{s + i}")
                            for i in range(len(sset))
                        ]
                        # zigzag tap order: adjacent sets share the boundary
                        # tap, so its LDWEIGHTS dedupes away.
                        taps = range(NTAP) if nset_state[0] % 2 == 0 else (
                            range(NTAP - 1, -1, -1)
                        )
                        nset_state[0] += 1
                        for tj, t in enumerate(taps):
                            kh, kw = divmod(t, 3)
                            toff = kh * SW + kw
                            chain(nc.tensor.ldweights(
                                wt[:, t * C : (t + 1) * C]))
                            for i, (c0, cl) in enumerate(sset):
                                loc = c0 - g0 + toff
                                chain(_mm_noldw(
                                    nc,
                                    pss[i][:, 0:cl],
                                    wt[:, t * C : (t + 1) * C],
                                    xt[:, loc : loc + cl],
                                    start=(tj == 0),
                                    stop=(tj == NTAP - 1),
                                ))
                        for i, (c0, cl) in enumerate(sset):
                            eng = nc.scalar.copy if i % 2 == 0 else (
                                nc.vector.tensor_copy
                            )
                            eng(ot[:, c0 - g0 : c0 - g0 + cl],
                                pss[i][:, 0:cl])
                    nc.sync.dma_start(
                        y_out[img, :, g0 : g0 + glen], ot[:, 0:glen]
                    )

            if repeat == 1:
                body()
            else:
                with tc.For_i(0, repeat, 1):
                    body()
    nc.compile()
    _dedupe_ldweights(nc)
    return nc


def _fold_weights(w_square, w_ver, w_hor, w_diag19, w_diag37):
    """Fold the 5 branches into one 3x3 weight, laid out [C_in, tap*C_out]."""
    eye = np.eye(3, dtype=np.float32)
    anti = eye[::-1, :]
    w_eff = (
        np.asarray(w_square, np.float32)
        + np.asarray(w_diag19, np.float32) * eye
        + np.asarray(w_diag37, np.float32) * anti
    )
    w_eff[:, :, :, 1] += np.asarray(w_ver, np.float32)[:, :, :, 0]
    w_eff[:, :, 1, :] += np.asarray(w_hor, np.float32)[:, :, 0, :]
    # [O, I, KH, KW] -> [I, KH, KW, O] -> [I, (KH*KW)*O]  (lhsT per tap)
    w = np.ascontiguousarray(w_eff.transpose(1, 2, 3, 0).reshape(C, NTAP * C))
    return w.astype(_np_bf16())


def _pack_x(x):
    """[B,C,H,W] -> spacer-packed flat bf16 [B,C,XLEN]."""
    xs = np.zeros((B, C, XLEN), _np_bf16())
    rows = xs[:, :, 1 : 1 + (H + 2) * SW].reshape(B, C, H + 2, SW)
    rows[:, :, 1 : H + 1, 0:W] = x.astype(_np_bf16())
    return xs


_nc_cache = {}


def build_program(repeat=1, **flags):
    """Canonical program used by kernel() and test.py."""
    return _build_flat(IPC, repeat=repeat, **flags)


def _unpack_y(y):
    """[B, C, ...] device output -> [B, C, H, W] fp32."""
    y = np.asarray(y)
    if y.shape[-1] == FLAT:  # spacer-packed flat layout
        y = y.reshape(B, C, H, SW)[:, :, :, 0:W]
    return np.ascontiguousarray(y.astype(np.float32))


def kernel(x, w_square, w_ver, w_hor, w_diag19, w_diag37):
    x = np.asarray(x, np.float32)
    w_host = _fold_weights(w_square, w_ver, w_hor, w_diag19, w_diag37)
    xs = _pack_x(x)

    if "nc" not in _nc_cache:
        _nc_cache["nc"] = build_program()
    nc = _nc_cache["nc"]

    in_maps = [
        {"x": np.ascontiguousarray(xs[c * IPC : (c + 1) * IPC]), "w": w_host}
        for c in range(NCORES)
    ]
    res = run_bass_kernel_spmd(nc, in_maps, list(range(NCORES)))
    out = np.concatenate([res.results[c]["y"] for c in range(NCORES)], axis=0)
    return _unpack_y(out)


# revision 31
# speedup vs baseline: 1.1029x; 1.0310x over previous
"""Fused ACNet-style 5-branch conv block as a single 3x3 conv on Trainium2.

The reference computes
    out = conv3x3(x, w_square) + conv3x1(x, w_ver) + conv1x3(x, w_hor)
        + conv3x3(x, w_diag19 * eye3) + conv3x3(x, w_diag37 * antieye3)
All five branches are linear convs with identical output geometry, so they
fold into ONE effective 3x3 conv whose weight is the sum of the embedded /
masked branch weights.  The conv runs as 9 shifted matmuls (one per tap)
accumulated in PSUM, channels on the 128 SBUF partitions (C_in = C_out = 128):
    out[:, h, w] += W[kh,kw].T @ x_pad[:, h+kh, w+kw]

Input layout: spacer-packed rows — each padded row is 193 elements (192 data
+ 1 shared zero spacer).  The spacer acts as right-pad of row r AND left-pad
of row r+1, so every tap shift is a pure flat offset and ANY flat window of
the packed image is a valid moving operand.  The packed output space
(H*193 per image) is tiled into 512-wide chunks = full PSUM banks
(_build_flat), minimizing matmul instruction count (73/image vs 96 for
row-pair tiling).

Operands are bf16 (host-converted): enables fast weight load and halves HBM
traffic; PSUM accumulates fp32, outputs drain to bf16 and the host converts
back to fp32 (rel err ~3.4e-3, tolerance 2e-2).  Matmuls are issued
tap-major across sets of 4 PSUM banks with ONE explicit LDWEIGHTS per tap
(matmuls carry ldweights=False; a post-compile pass drops the redundant
auto-inserted weight reloads — ~70us win at fp32->bf16+dedupe combined).
Tap order zigzags between sets so boundary taps share weight loads.
Outputs stay spacer-packed in DRAM; the host strips the 193rd columns.

Sharding: data-parallel over batch — 16 images / 8 cores = 2 images per
core, weights replicated, no collectives.
"""

import sys

for _p in ("/opt/trn_rl_repo",):
    if _p not in sys.path:
        sys.path.insert(0, _p)

import numpy as np

import concourse.mybir as mybir
import concourse.tile as tile
from concourse import bacc
from concourse.bass_utils import run_bass_kernel_spmd
from concourse.tile_rust import add_dep_helper

B, C, H, W = 16, 128, 192, 192
NCORES = 8
IPC = B // NCORES  # images per core
NTAP = 9
SW = W + 1  # spacer-packed row width (193)
XLEN = 1 + (H + 2) * SW + 4  # leading zero + 194 packed rows + tap margin
RB = 32  # output rows per block
GSET = 4  # 2-row groups per PSUM bank set (tap-major inner tile)
MM_DT = mybir.dt.bfloat16

_bf16 = None


def _np_bf16():
    global _bf16
    if _bf16 is None:
        _bf16 = mybir.dt.np(mybir.dt.bfloat16)
    return _bf16


def _mm_noldw(nc, out, lhsT, rhs, start, stop):
    """InstMatmult with ldweights=False: uses the weights already loaded
    into the PE array by a preceding explicit nc.tensor.ldweights."""
    te = nc.tensor
    ifmap_ap = te.lower_ap(rhs.opt({0}), opt=False)
    weights_ap = te.lower_ap(lhsT.opt({0}), opt=False, for_matmul_weights=True)
    out_ap = te.lower_ap(out)
    return te.add_instruction(
        mybir.InstMatmult(
            name=te.bass.get_next_instruction_name(),
            replication_resolution=0,
            replication_shift_amnt=0,
            replication_num_rows=0,
            start_tensor_calc=start,
            stop_tensor_calc=stop,
            ins=[ifmap_ap, weights_ap],
            outs=[out_ap],
            perf_mode=None,
            is_transpose=None,
            ifmap_quant_offset=None,
            weights_quant_offset=None,
            bass_skip_group_check=False,
            tile_position=(0, 0),
            tile_size=(128, 128),
            ldweights=False,
        )
    )


def _dedupe_ldweights(nc):
    """Drop InstLdweights that reload the weights already resident in the
    PE array (identical AP to the previous load, no different load in
    between).  The compile pipeline emits one LDWEIGHTS per matmul even for
    ldweights=False matmuls; within a tap-major run they are redundant.
    Waits/updates on a dropped LDW migrate to the next kept PE instruction.
    """
    for bb in nc.m.functions[0].blocks:
        il = bb.instructions
        keep = []
        last_key = None
        for inst in il:
            if isinstance(inst, mybir.InstLdweights):
                key = str(inst.ins[0])
                si = inst.sync_info
                has_sync = si is not None and (
                    len(si.on_wait) > 0 or len(si.on_update) > 0
                )
                if key == last_key and not has_sync:
                    continue  # redundant reload with no sync riding: drop
                last_key = key
            keep.append(inst)
        il[:] = keep


def _build(ipc, rb, mm_dt, repeat=1, xbufs=3, obufs=2, ahead=1, gset=GSET,
           do_mm=True, do_in=True, do_out=True, do_drain=True, psum2d=True):
    """Emit the per-core Bass program.

    The x-DMA for block k+ahead is issued before block k's compute/out-DMA
    in program order, so input prefetch never queues behind output drains.
    repeat>1 wraps the body in a For_i loop (timing harness only; the body
    is idempotent so outputs are unchanged).
    """
    nc = bacc.Bacc("TRN2", target_bir_lowering=False, debug=False)
    x_in = nc.dram_tensor(
        "x", [ipc, C, XLEN], mm_dt, kind="ExternalInput"
    ).ap()
    w_in = nc.dram_tensor(
        "w", [C, NTAP * C], mm_dt, kind="ExternalInput"
    ).ap()
    y_out = nc.dram_tensor(
        "y", [ipc, C, H, W], mm_dt, kind="ExternalOutput"
    ).ap()

    xtl = (rb + 2) * SW + 4  # x tile flat length per partition
    blocks = [(img, r0) for img in range(ipc) for r0 in range(0, H, rb)]

    with tile.TileContext(nc) as tc:
        with (
            tc.tile_pool(name="wp", bufs=1) as wpool,
            tc.tile_pool(name="xp", bufs=xbufs) as xpool,
            tc.tile_pool(name="op", bufs=obufs) as opool,
            tc.tile_pool(name="ps", bufs=8, space="PSUM") as pspool,
        ):
            wt = wpool.tile([C, NTAP * C], mm_dt)
            nc.sync.dma_start(wt[:], w_in[:])

            def load(img, r0):
                xt = xpool.tile([C, xtl], mm_dt, tag="xt", name=f"xt{img}_{r0}")
                base = r0 * SW
                if do_in:
                    nc.sync.dma_start(xt[:], x_in[img, :, base : base + xtl])
                return xt

            prev_pe = [None]  # last PE instruction, for the order chain

            def chain(bi):
                # Serialize the PE stream in program order: the scheduler
                # can't see the PE weight-state hazard between a standalone
                # ldweights and the non-self-loading matmuls that use it.
                if prev_pe[0] is not None:
                    add_dep_helper(bi.ins, prev_pe[0].ins, sync=False,
                                   reason="pe-weight-state order")
                prev_pe[0] = bi

            def body():
                xts = [load(*blocks[k]) for k in range(min(ahead, len(blocks)))]
                for k, (img, r0) in enumerate(blocks):
                    if k + ahead < len(blocks):
                        xts.append(load(*blocks[k + ahead]))
                    xt = xts.pop(0)
                    ot = opool.tile([C, rb, W], mm_dt, tag="ot",
                                    name=f"ot{img}_{r0}")
                    ngroups = rb // 2
                    for s in range(0, ngroups, gset):
                        nset = min(gset, ngroups - s)
                        shape = [C, 2, SW] if psum2d else [C, 2 * SW]
                        pss = [
                            pspool.tile(shape, mybir.dt.float32,
                                        tag="ps", name=f"ps{s + i}")
                            for i in range(nset)
                        ]
                        # tap-major with one explicit weight load per tap,
                        # amortized over the whole bank set.
                        if do_mm:
                            for t in range(NTAP):
                                kh, kw = divmod(t, 3)
                                chain(nc.tensor.ldweights(
                                    wt[:, t * C : (t + 1) * C]))
                                for i in range(nset):
                                    p = s + i
                                    off = (2 * p + kh) * SW + kw
                                    chain(_mm_noldw(
                                        nc,
                                        pss[i][:],
                                        wt[:, t * C : (t + 1) * C],
                                        xt[:, off : off + 2 * SW],
                                        start=(t == 0),
                                        stop=(t == NTAP - 1),
                                    ))
                        # strip the spacer columns while draining PSUM
                        # (one strided 2-row op per bank)
                        if do_drain and do_mm:
                            for i in range(nset):
                                p = s + i
                                eng = nc.scalar.copy if i % 2 == 0 else (
                                    nc.vector.tensor_copy
                                )
                                if psum2d:
                                    eng(ot[:, 2 * p : 2 * p + 2, :],
                                        pss[i][:, :, 0:W])
                                else:
                                    eng(ot[:, 2 * p, :], pss[i][:, 0:W])
                                    eng(ot[:, 2 * p + 1, :],
                                        pss[i][:, SW : SW + W])
                    if do_out:
                        nc.sync.dma_start(
                            y_out[img, :, r0 : r0 + rb, :], ot[:])

            if repeat == 1:
                body()
            else:
                with tc.For_i(0, repeat, 1):
                    body()
    nc.compile()
    _dedupe_ldweights(nc)
    return nc


FLAT = H * SW  # packed output length per image (192 rows x 193)
CHUNK = 512  # matmul free dim / PSUM bank
XGRP = 16  # chunks per x-load group
PSET = 4  # chunks per PSUM bank set (tap-major inner tile)


def _chunks_of(img_len):
    """Chunk starts+lengths covering [0, img_len) in 512s + remainder."""
    out = []
    c0 = 0
    while c0 < img_len:
        out.append((c0, min(CHUNK, img_len - c0)))
        c0 += CHUNK
    return out


def _build_flat(ipc, repeat=1, xbufs=3, obufs=2, ahead=1, pset=PSET,
                mm_dt=MM_DT):
    """Flat-chunk variant: the packed per-image output space [0, H*SW) is
    tiled into 512-wide matmul chunks (full PSUM banks), ignoring row
    boundaries — a tap shift is a pure flat offset, so any flat window is a
    valid moving operand.  Outputs stay spacer-packed ([C, H*SW] bf16 in
    DRAM); the host strips the 193rd column of each row.

    x is streamed in groups of 16 chunks (+390-element tap halo), output
    drains into 2-superset (1 MiB) tiles.
    """
    nc = bacc.Bacc("TRN2", target_bir_lowering=False, debug=False)
    x_in = nc.dram_tensor(
        "x", [ipc, C, XLEN], mm_dt, kind="ExternalInput"
    ).ap()
    w_in = nc.dram_tensor(
        "w", [C, NTAP * C], mm_dt, kind="ExternalInput"
    ).ap()
    y_out = nc.dram_tensor(
        "y", [ipc, C, FLAT], mm_dt, kind="ExternalOutput"
    ).ap()

    halo = 2 * SW + 2  # max tap offset reach
    chunks = _chunks_of(FLAT)
    # First group small so the PE starts after ~0.6MB of x instead of 2.1MB;
    # small tail group keeps the final drain+store burst short.
    sizes = [4] + [XGRP] * ((len(chunks) - 4) // XGRP)
    rem = len(chunks) - sum(sizes)
    if rem:
        sizes.append(rem)
    groups, pos = [], 0
    for sz in sizes:
        groups.append(chunks[pos : pos + sz])
        pos += sz
    loads = []  # (img, g0, xlen, group-chunks)
    for img in range(ipc):
        for grp in groups:
            g0 = grp[0][0]
            gend = grp[-1][0] + grp[-1][1]
            loads.append((img, g0, min(gend + halo, XLEN) - g0, grp))

    with tile.TileContext(nc) as tc:
        with (
            tc.tile_pool(name="wp", bufs=1) as wpool,
            tc.tile_pool(name="xp", bufs=xbufs) as xpool,
            tc.tile_pool(name="op", bufs=obufs) as opool,
            tc.tile_pool(name="ps", bufs=8, space="PSUM") as pspool,
        ):
            wt = wpool.tile([C, NTAP * C], mm_dt)
            nc.sync.dma_start(wt[:], w_in[:])

            xtl = XGRP * CHUNK + halo
            prev_pe = [None]

            def chain(bi):
                if prev_pe[0] is not None:
                    add_dep_helper(bi.ins, prev_pe[0].ins, sync=False,
                                   reason="pe-weight-state order")
                prev_pe[0] = bi

            def load(k):
                img, g0, xl, grp = loads[k]
                xt = xpool.tile([C, xtl], mm_dt, tag="xt", name=f"xt{k}")
                nc.sync.dma_start(xt[:, 0:xl], x_in[img, :, g0 : g0 + xl])
                return xt

            nset_state = [0]

            def body():
                xts = [load(k) for k in range(min(ahead, len(loads)))]
                for k, (img, g0, xl, grp) in enumerate(loads):
                    if k + ahead < len(loads):
                        xts.append(load(k + ahead))
                    xt = xts.pop(0)
                    glen = grp[-1][0] + grp[-1][1] - g0
                    ot = opool.tile([C, XGRP * CHUNK], mm_dt, tag="ot",
                                    name=f"ot{k}")
                    drained = 0
                    flushed = 0
                    # split the group's chunks into near-even bank sets
                    # (avoids a singleton tail set paying 9 LDWs for 1 MM)
                    nsets = max(1, round(len(grp) / pset))
                    bounds = [len(grp) * j // nsets for j in range(nsets + 1)]
                    for s, send in zip(bounds[:-1], bounds[1:]):
                        sset = grp[s:send]
                        pss = [
                            pspool.tile([C, CHUNK], mybir.dt.float32,
                                        tag="ps", name=f"ps{k}_{s + i}")
                            for i in range(len(sset))
                        ]
                        # zigzag tap order: adjacent sets share the boundary
                        # tap, so its LDWEIGHTS dedupes away.
                        taps = range(NTAP) if nset_state[0] % 2 == 0 else (
                            range(NTAP - 1, -1, -1)
                        )
                        nset_state[0] += 1
                        for tj, t in enumerate(taps):
                            kh, kw = divmod(t, 3)
                            toff = kh * SW + kw
                            chain(nc.tensor.ldweights(
                                wt[:, t * C : (t + 1) * C]))
                            for i, (c0, cl) in enumerate(sset):
                                loc = c0 - g0 + toff
                                chain(_mm_noldw(
                                    nc,
                                    pss[i][:, 0:cl],
                                    wt[:, t * C : (t + 1) * C],
                                    xt[:, loc : loc + cl],
                                    start=(tj == 0),
                                    stop=(tj == NTAP - 1),
                                ))
                        for i, (c0, cl) in enumerate(sset):
                            eng = nc.scalar.copy if i % 2 == 0 else (
                                nc.vector.tensor_copy
                            )
                            eng(ot[:, c0 - g0 : c0 - g0 + cl],
                                pss[i][:, 0:cl])
                        # flush drained output in ~1MiB stores so the final
                        # store overlaps later supersets' compute
                        drained += len(sset)
                        if drained >= 8 or send >= len(grp):
                            end = sset[-1][0] + sset[-1][1] - g0
                            nc.sync.dma_start(
                                y_out[img, :, g0 + flushed : g0 + end],
                                ot[:, flushed:end],
                            )
                            flushed = end
                            drained = 0

            if repeat == 1:
                body()
            else:
                with tc.For_i(0, repeat, 1):
                    body()
    nc.compile()
    _dedupe_ldweights(nc)
    return nc


def _fold_weights(w_square, w_ver, w_hor, w_diag19, w_diag37):
    """Fold the 5 branches into one 3x3 weight, laid out [C_in, tap*C_out]."""
    eye = np.eye(3, dtype=np.float32)
    anti = eye[::-1, :]
    w_eff = (
        np.asarray(w_square, np.float32)
        + np.asarray(w_diag19, np.float32) * eye
        + np.asarray(w_diag37, np.float32) * anti
    )
    w_eff[:, :, :, 1] += np.asarray(w_ver, np.float32)[:, :, :, 0]
    w_eff[:, :, 1, :] += np.asarray(w_hor, np.float32)[:, :, 0, :]
    # [O, I, KH, KW] -> [I, KH, KW, O] -> [I, (KH*KW)*O]  (lhsT per tap)
    w = np.ascontiguousarray(w_eff.transpose(1, 2, 3, 0).reshape(C, NTAP * C))
    return w.astype(_np_bf16())


def _pack_x(x):
    """[B,C,H,W] -> spacer-packed flat bf16 [B,C,XLEN]."""
    xs = np.zeros((B, C, XLEN), _np_bf16())
    rows = xs[:, :, 1 : 1 + (H + 2) * SW].reshape(B, C, H + 2, SW)
    rows[:, :, 1 : H + 1, 0:W] = x.astype(_np_bf16())
    return xs


_nc_cache = {}


def build_program(repeat=1, **flags):
    """Canonical program used by kernel() and test.py."""
    return _build_flat(IPC, repeat=repeat, **flags)


def _unpack_y(y):
    """[B, C, ...] device output -> [B, C, H, W] fp32."""
    y = np.asarray(y)
    if y.shape[-1] == FLAT:  # spacer-packed flat layout
        y = y.reshape(B, C, H, SW)[:, :, :, 0:W]
    return np.ascontiguousarray(y.astype(np.float32))


def kernel(x, w_square, w_ver, w_hor, w_diag19, w_diag37):
    x = np.asarray(x, np.float32)
    w_host = _fold_weights(w_square, w_ver, w_hor, w_diag19, w_diag37)
    xs = _pack_x(x)

    if "nc" not in _nc_cache:
        _nc_cache["nc"] = build_program()
    nc = _nc_cache["nc"]

    in_maps = [
        {"x": np.ascontiguousarray(xs[c * IPC : (c + 1) * IPC]), "w": w_host}
        for c in range(NCORES)
    ]
    res = run_bass_kernel_spmd(nc, in_maps, list(range(NCORES)))
    out = np.concatenate([res.results[c]["y"] for c in range(NCORES)], axis=0)
    return _unpack_y(out)


# revision 34
# speedup vs baseline: 1.1281x; 1.0229x over previous
"""Fused ACNet-style 5-branch conv block as a single 3x3 conv on Trainium2.

The reference computes
    out = conv3x3(x, w_square) + conv3x1(x, w_ver) + conv1x3(x, w_hor)
        + conv3x3(x, w_diag19 * eye3) + conv3x3(x, w_diag37 * antieye3)
All five branches are linear convs with identical output geometry, so they
fold into ONE effective 3x3 conv whose weight is the sum of the embedded /
masked branch weights.  The conv runs as 9 shifted matmuls (one per tap)
accumulated in PSUM, channels on the 128 SBUF partitions (C_in = C_out = 128):
    out[:, h, w] += W[kh,kw].T @ x_pad[:, h+kh, w+kw]

Input layout: spacer-packed rows — each padded row is 193 elements (192 data
+ 1 shared zero spacer).  The spacer acts as right-pad of row r AND left-pad
of row r+1, so every tap shift is a pure flat offset and ANY flat window of
the packed image is a valid moving operand.  The packed output space
(H*193 per image) is tiled into 512-wide chunks = full PSUM banks
(_build_flat), minimizing matmul instruction count (73/image vs 96 for
row-pair tiling).

Operands are bf16 (host-converted): enables fast weight load and halves HBM
traffic; PSUM accumulates fp32, outputs drain to bf16 and the host converts
back to fp32 (rel err ~3.4e-3, tolerance 2e-2).  Matmuls are issued
tap-major across sets of 4 PSUM banks with ONE explicit LDWEIGHTS per tap
(matmuls carry ldweights=False; a post-compile pass drops the redundant
auto-inserted weight reloads — ~70us win at fp32->bf16+dedupe combined).
Tap order zigzags between sets so boundary taps share weight loads.
Outputs stay spacer-packed in DRAM; the host strips the 193rd columns.

Sharding: data-parallel over batch — 16 images / 8 cores = 2 images per
core, weights replicated, no collectives.
"""

import sys

for _p in ("/opt/trn_rl_repo",):
    if _p not in sys.path:
        sys.path.insert(0, _p)

import numpy as np

import concourse.mybir as mybir
import concourse.tile as tile
from concourse import bacc
from concourse.bass_utils import run_bass_kernel_spmd
from concourse.tile_rust import add_dep_helper

B, C, H, W = 16, 128, 192, 192
NCORES = 8
IPC = B // NCORES  # images per core
NTAP = 9
SW = W + 1  # spacer-packed row width (193)
XLEN = 1 + (H + 2) * SW + 4  # leading zero + 194 packed rows + tap margin
RB = 32  # output rows per block
GSET = 4  # 2-row groups per PSUM bank set (tap-major inner tile)
MM_DT = mybir.dt.bfloat16

_bf16 = None


def _np_bf16():
    global _bf16
    if _bf16 is None:
        _bf16 = mybir.dt.np(mybir.dt.bfloat16)
    return _bf16


def _mm_noldw(nc, out, lhsT, rhs, start, stop,
              tile_position=(0, 0), tile_size=(128, 128)):
    """InstMatmult with ldweights=False: uses the weights already loaded
    into the PE array by a preceding explicit nc.tensor.ldweights."""
    te = nc.tensor
    ifmap_ap = te.lower_ap(rhs.opt({0}), opt=False)
    weights_ap = te.lower_ap(lhsT.opt({0}), opt=False, for_matmul_weights=True)
    out_ap = te.lower_ap(out)
    return te.add_instruction(
        mybir.InstMatmult(
            name=te.bass.get_next_instruction_name(),
            replication_resolution=0,
            replication_shift_amnt=0,
            replication_num_rows=0,
            start_tensor_calc=start,
            stop_tensor_calc=stop,
            ins=[ifmap_ap, weights_ap],
            outs=[out_ap],
            perf_mode=None,
            is_transpose=None,
            ifmap_quant_offset=None,
            weights_quant_offset=None,
            bass_skip_group_check=False,
            tile_position=tile_position,
            tile_size=tile_size,
            ldweights=False,
        )
    )


def _dedupe_ldweights(nc):
    """Drop InstLdweights that reload the weights already resident in the
    PE array (identical AP to the previous load, no different load in
    between).  The compile pipeline emits one LDWEIGHTS per matmul even for
    ldweights=False matmuls; within a tap-major run they are redundant.
    Waits/updates on a dropped LDW migrate to the next kept PE instruction.
    """
    for bb in nc.m.functions[0].blocks:
        il = bb.instructions
        keep = []
        last_key = None
        for inst in il:
            if isinstance(inst, mybir.InstLdweights):
                key = str(inst.ins[0])
                si = inst.sync_info
                has_sync = si is not None and (
                    len(si.on_wait) > 0 or len(si.on_update) > 0
                )
                if key == last_key and not has_sync:
                    continue  # redundant reload with no sync riding: drop
                last_key = key
            keep.append(inst)
        il[:] = keep


def _build(ipc, rb, mm_dt, repeat=1, xbufs=3, obufs=2, ahead=1, gset=GSET,
           do_mm=True, do_in=True, do_out=True, do_drain=True, psum2d=True):
    """Emit the per-core Bass program.

    The x-DMA for block k+ahead is issued before block k's compute/out-DMA
    in program order, so input prefetch never queues behind output drains.
    repeat>1 wraps the body in a For_i loop (timing harness only; the body
    is idempotent so outputs are unchanged).
    """
    nc = bacc.Bacc("TRN2", target_bir_lowering=False, debug=False)
    x_in = nc.dram_tensor(
        "x", [ipc, C, XLEN], mm_dt, kind="ExternalInput"
    ).ap()
    w_in = nc.dram_tensor(
        "w", [C, NTAP * C], mm_dt, kind="ExternalInput"
    ).ap()
    y_out = nc.dram_tensor(
        "y", [ipc, C, H, W], mm_dt, kind="ExternalOutput"
    ).ap()

    xtl = (rb + 2) * SW + 4  # x tile flat length per partition
    blocks = [(img, r0) for img in range(ipc) for r0 in range(0, H, rb)]

    with tile.TileContext(nc) as tc:
        with (
            tc.tile_pool(name="wp", bufs=1) as wpool,
            tc.tile_pool(name="xp", bufs=xbufs) as xpool,
            tc.tile_pool(name="op", bufs=obufs) as opool,
            tc.tile_pool(name="ps", bufs=8, space="PSUM") as pspool,
        ):
            wt = wpool.tile([C, NTAP * C], mm_dt)
            nc.sync.dma_start(wt[:], w_in[:])

            def load(img, r0):
                xt = xpool.tile([C, xtl], mm_dt, tag="xt", name=f"xt{img}_{r0}")
                base = r0 * SW
                if do_in:
                    nc.sync.dma_start(xt[:], x_in[img, :, base : base + xtl])
                return xt

            prev_pe = [None]  # last PE instruction, for the order chain

            def chain(bi):
                # Serialize the PE stream in program order: the scheduler
                # can't see the PE weight-state hazard between a standalone
                # ldweights and the non-self-loading matmuls that use it.
                if prev_pe[0] is not None:
                    add_dep_helper(bi.ins, prev_pe[0].ins, sync=False,
                                   reason="pe-weight-state order")
                prev_pe[0] = bi

            def body():
                xts = [load(*blocks[k]) for k in range(min(ahead, len(blocks)))]
                for k, (img, r0) in enumerate(blocks):
                    if k + ahead < len(blocks):
                        xts.append(load(*blocks[k + ahead]))
                    xt = xts.pop(0)
                    ot = opool.tile([C, rb, W], mm_dt, tag="ot",
                                    name=f"ot{img}_{r0}")
                    ngroups = rb // 2
                    for s in range(0, ngroups, gset):
                        nset = min(gset, ngroups - s)
                        shape = [C, 2, SW] if psum2d else [C, 2 * SW]
                        pss = [
                            pspool.tile(shape, mybir.dt.float32,
                                        tag="ps", name=f"ps{s + i}")
                            for i in range(nset)
                        ]
                        # tap-major with one explicit weight load per tap,
                        # amortized over the whole bank set.
                        if do_mm:
                            for t in range(NTAP):
                                kh, kw = divmod(t, 3)
                                chain(nc.tensor.ldweights(
                                    wt[:, t * C : (t + 1) * C]))
                                for i in range(nset):
                                    p = s + i
                                    off = (2 * p + kh) * SW + kw
                                    chain(_mm_noldw(
                                        nc,
                                        pss[i][:],
                                        wt[:, t * C : (t + 1) * C],
                                        xt[:, off : off + 2 * SW],
                                        start=(t == 0),
                                        stop=(t == NTAP - 1),
                                    ))
                        # strip the spacer columns while draining PSUM
                        # (one strided 2-row op per bank)
                        if do_drain and do_mm:
                            for i in range(nset):
                                p = s + i
                                eng = nc.scalar.copy if i % 2 == 0 else (
                                    nc.vector.tensor_copy
                                )
                                if psum2d:
                                    eng(ot[:, 2 * p : 2 * p + 2, :],
                                        pss[i][:, :, 0:W])
                                else:
                                    eng(ot[:, 2 * p, :], pss[i][:, 0:W])
                                    eng(ot[:, 2 * p + 1, :],
                                        pss[i][:, SW : SW + W])
                    if do_out:
                        nc.sync.dma_start(
                            y_out[img, :, r0 : r0 + rb, :], ot[:])

            if repeat == 1:
                body()
            else:
                with tc.For_i(0, repeat, 1):
                    body()
    nc.compile()
    _dedupe_ldweights(nc)
    return nc


FLAT = H * SW  # packed output length per image (192 rows x 193)
CHUNK = 512  # matmul free dim / PSUM bank
XGRP = 16  # chunks per x-load group
PSET = 4  # chunks per PSUM bank set (tap-major inner tile)


def _chunks_of(img_len):
    """Chunk starts+lengths covering [0, img_len) in 512s + remainder."""
    out = []
    c0 = 0
    while c0 < img_len:
        out.append((c0, min(CHUNK, img_len - c0)))
        c0 += CHUNK
    return out


def _build_flat(ipc, repeat=1, xbufs=3, obufs=2, ahead=1, pset=PSET,
                mm_dt=MM_DT, ksplit=False):
    """Flat-chunk variant: the packed per-image output space [0, H*SW) is
    tiled into 512-wide matmul chunks (full PSUM banks), ignoring row
    boundaries — a tap shift is a pure flat offset, so any flat window is a
    valid moving operand.  Outputs stay spacer-packed ([C, H*SW] bf16 in
    DRAM); the host strips the 193rd column of each row.

    x is streamed in groups of 16 chunks (+390-element tap halo), output
    drains into 2-superset (1 MiB) tiles.
    """
    nc = bacc.Bacc("TRN2", target_bir_lowering=False, debug=False)
    x_in = nc.dram_tensor(
        "x", [ipc, C, XLEN], mm_dt, kind="ExternalInput"
    ).ap()
    w_in = nc.dram_tensor(
        "w", [C, NTAP * C], mm_dt, kind="ExternalInput"
    ).ap()
    y_out = nc.dram_tensor(
        "y", [ipc, C, FLAT], mm_dt, kind="ExternalOutput"
    ).ap()

    halo = 2 * SW + 2  # max tap offset reach
    chunks = _chunks_of(FLAT)
    # First group small so the PE starts after ~0.6MB of x instead of 2.1MB;
    # small tail group keeps the final drain+store burst short.
    sizes = [4] + [XGRP] * ((len(chunks) - 4) // XGRP)
    rem = len(chunks) - sum(sizes)
    if rem:
        sizes.append(rem)
    groups, pos = [], 0
    for sz in sizes:
        groups.append(chunks[pos : pos + sz])
        pos += sz
    loads = []  # (img, g0, xlen, group-chunks)
    for img in range(ipc):
        for grp in groups:
            g0 = grp[0][0]
            gend = grp[-1][0] + grp[-1][1]
            loads.append((img, g0, min(gend + halo, XLEN) - g0, grp))

    with tile.TileContext(nc) as tc:
        with (
            tc.tile_pool(name="wp", bufs=1) as wpool,
            tc.tile_pool(name="xp", bufs=xbufs) as xpool,
            tc.tile_pool(name="op", bufs=obufs) as opool,
            tc.tile_pool(name="ps", bufs=8, space="PSUM") as pspool,
        ):
            wt = wpool.tile([C, NTAP * C], mm_dt)
            nc.sync.dma_start(wt[:], w_in[:])

            xtl = XGRP * CHUNK + halo
            prev_pe = [None]

            def chain(bi):
                if prev_pe[0] is not None:
                    add_dep_helper(bi.ins, prev_pe[0].ins, sync=False,
                                   reason="pe-weight-state order")
                prev_pe[0] = bi

            def load(k):
                img, g0, xl, grp = loads[k]
                xt = xpool.tile([C, xtl], mm_dt, tag="xt", name=f"xt{k}")
                nc.sync.dma_start(xt[:, 0:xl], x_in[img, :, g0 : g0 + xl])
                return xt

            nset_state = [0]

            def body():
                xts = [load(k) for k in range(min(ahead, len(loads)))]
                for k, (img, g0, xl, grp) in enumerate(loads):
                    if k + ahead < len(loads):
                        xts.append(load(k + ahead))
                    xt = xts.pop(0)
                    glen = grp[-1][0] + grp[-1][1] - g0
                    ot = opool.tile([C, XGRP * CHUNK], mm_dt, tag="ot",
                                    name=f"ot{k}")
                    drained = 0
                    flushed = 0
                    # split the group's chunks into near-even bank sets
                    # (avoids a singleton tail set paying 9 LDWs for 1 MM)
                    nsets = max(1, round(len(grp) / pset))
                    bounds = [len(grp) * j // nsets for j in range(nsets + 1)]
                    for s, send in zip(bounds[:-1], bounds[1:]):
                        sset = grp[s:send]
                        pss = [
                            pspool.tile([C, CHUNK], mybir.dt.float32,
                                        tag="ps", name=f"ps{k}_{s + i}")
                            for i in range(len(sset))
                        ]
                        # zigzag tap order: adjacent sets share the boundary
                        # tap, so its LDWEIGHTS dedupes away.
                        taps = range(NTAP) if nset_state[0] % 2 == 0 else (
                            range(NTAP - 1, -1, -1)
                        )
                        nset_state[0] += 1
                        for tj, t in enumerate(taps):
                            kh, kw = divmod(t, 3)
                            toff = kh * SW + kw
                            if not ksplit:
                                chain(nc.tensor.ldweights(
                                    wt[:, t * C : (t + 1) * C]))
                                for i, (c0, cl) in enumerate(sset):
                                    loc = c0 - g0 + toff
                                    chain(_mm_noldw(
                                        nc,
                                        pss[i][:, 0:cl],
                                        wt[:, t * C : (t + 1) * C],
                                        xt[:, loc : loc + cl],
                                        start=(tj == 0),
                                        stop=(tj == NTAP - 1),
                                    ))
                                continue
                            # K=64 row-split: halves alternate so each
                            # half's LDWEIGHTS is pulled ahead by HW while
                            # the OTHER half's matmuls stream (row groups
                            # don't conflict).
                            for h, k0 in enumerate((0, 64)):
                                chain(nc.tensor.ldweights(
                                    wt[k0 : k0 + 64, t * C : (t + 1) * C],
                                    tile_position=(k0, 0),
                                ))
                                for i, (c0, cl) in enumerate(sset):
                                    loc = c0 - g0 + toff
                                    chain(_mm_noldw(
                                        nc,
                                        pss[i][:, 0:cl],
                                        wt[k0 : k0 + 64,
                                           t * C : (t + 1) * C],
                                        xt[k0 : k0 + 64, loc : loc + cl],
                                        start=(tj == 0 and h == 0),
                                        stop=(tj == NTAP - 1 and h == 1),
                                        tile_position=(k0, 0),
                                        tile_size=(64, 128),
                                    ))
                        for i, (c0, cl) in enumerate(sset):
                            eng = nc.scalar.copy if i % 2 == 0 else (
                                nc.vector.tensor_copy
                            )
                            eng(ot[:, c0 - g0 : c0 - g0 + cl],
                                pss[i][:, 0:cl])
                        # flush drained output in ~1MiB stores so the final
                        # store overlaps later supersets' compute
                        drained += len(sset)
                        if drained >= 8 or send >= len(grp):
                            end = sset[-1][0] + sset[-1][1] - g0
                            nc.sync.dma_start(
                                y_out[img, :, g0 + flushed : g0 + end],
                                ot[:, flushed:end],
                            )
                            flushed = end
                            drained = 0

            if repeat == 1:
                body()
            else:
                with tc.For_i(0, repeat, 1):
                    body()
    nc.compile()
    _dedupe_ldweights(nc)
    return nc


def _fold_weights(w_square, w_ver, w_hor, w_diag19, w_diag37):
    """Fold the 5 branches into one 3x3 weight, laid out [C_in, tap*C_out]."""
    eye = np.eye(3, dtype=np.float32)
    anti = eye[::-1, :]
    w_eff = (
        np.asarray(w_square, np.float32)
        + np.asarray(w_diag19, np.float32) * eye
        + np.asarray(w_diag37, np.float32) * anti
    )
    w_eff[:, :, :, 1] += np.asarray(w_ver, np.float32)[:, :, :, 0]
    w_eff[:, :, 1, :] += np.asarray(w_hor, np.float32)[:, :, 0, :]
    # [O, I, KH, KW] -> [I, KH, KW, O] -> [I, (KH*KW)*O]  (lhsT per tap)
    w = np.ascontiguousarray(w_eff.transpose(1, 2, 3, 0).reshape(C, NTAP * C))
    return w.astype(_np_bf16())


def _pack_x(x):
    """[B,C,H,W] -> spacer-packed flat bf16 [B,C,XLEN]."""
    xs = np.zeros((B, C, XLEN), _np_bf16())
    rows = xs[:, :, 1 : 1 + (H + 2) * SW].reshape(B, C, H + 2, SW)
    rows[:, :, 1 : H + 1, 0:W] = x.astype(_np_bf16())
    return xs


_nc_cache = {}


def build_program(repeat=1, **flags):
    """Canonical program used by kernel() and test.py."""
    return _build_flat(IPC, repeat=repeat, **flags)


def _unpack_y(y):
    """[B, C, ...] device output -> [B, C, H, W] fp32."""
    y = np.asarray(y)
    if y.shape[-1] == FLAT:  # spacer-packed flat layout
        y = y.reshape(B, C, H, SW)[:, :, :, 0:W]
    return np.ascontiguousarray(y.astype(np.float32))


def kernel(x, w_square, w_ver, w_hor, w_diag19, w_diag37):
    x = np.asarray(x, np.float32)
    w_host = _fold_weights(w_square, w_ver, w_hor, w_diag19, w_diag37)
    xs = _pack_x(x)

    if "nc" not in _nc_cache:
        _nc_cache["nc"] = build_program()
    nc = _nc_cache["nc"]

    in_maps = [
        {"x": np.ascontiguousarray(xs[c * IPC : (c + 1) * IPC]), "w": w_host}
        for c in range(NCORES)
    ]
    res = run_bass_kernel_spmd(nc, in_maps, list(range(NCORES)))
    out = np.concatenate([res.results[c]["y"] for c in range(NCORES)], axis=0)
    return _unpack_y(out)
